# revision 8
# baseline (speedup 1.0000x reference)
"""Trainium2 Bass kernel for nn_Net_83794811945603 (3-layer GraphSAGE, mean agg).

Computation (N=50000 nodes, E=800000 edges):
    h0 = x @ W_map + b_map                                  [N,128]
    h1 = relu(mean_agg(h0) @ Wl1 + bl1 + h0 @ Wr1)          [N,128]
    h2 = relu(mean_agg(h1) @ Wl2 + bl2 + h1 @ Wr2)          [N,256]
    out = log_softmax(mean_agg(h2) @ Wl3 + bl3 + h2 @ Wr3)  [N,40]
where mean_agg(h)[i] = mean over edges (s->i) of h[s].

Strategy (8 NeuronCores, SPMD), v2:
  - Nodes sharded row-wise: core c owns nodes [c*6250, (c+1)*6250) and all
    edges whose dst lands there.  Weights replicated.
  - Per layer: full previous-layer features live in DRAM, produced by
    AllGather in FOUR tile-aligned chunks issued mid-loop so collectives
    overlap the remaining tile compute; next layer's gathers for chunk q
    start as soon as AG chunk q lands.
  - Edge-gather h[src] rows via SWDGE dma_gather (int16 indices relative to
    the chunk's table region).  Edges bucketed per (dst-tile, src-chunk);
    slot padding uses idx=-1 which the descriptor generator SKIPS (pads cost
    no DMA), with garbage slots neutralized by S=0 in the one-hot matmul.
  - Segment-sum by dst via TensorE one-hot matmuls: aggT += G_chunk^T @
    S_chunk, S built on VectorE as is_equal(iota, dl) in bf16.  The program
    is specialized to the exact per-bucket chunk counts (compiled per input
    distribution, cached by its signature).
  - Layer 3 aggregates z3 = h2 @ Wl3 (40->pad 64) instead of h2: 4x less
    gather traffic.  All dense matmuls run transposed ([feat, node]).
"""

import os
import sys

sys.path.insert(0, "/opt/trn_rl_repo")

import numpy as np
import ml_dtypes

import concourse.bass as bass
import concourse.bacc as bacc
import concourse.tile as tile
import concourse.mybir as mybir
from concourse.bass_utils import run_bass_kernel_spmd

F32 = mybir.dt.float32
BF16 = mybir.dt.bfloat16
I16 = mybir.dt.int16

N = 50000
E = 800000
F_IN = 500
F_IN_PAD = 512
D = 128            # ID_DIM == HID
D2 = 256           # 2*HID
NCLS = 40
W64 = 64           # padded class dim
NCORES = 8
NLOC = N // NCORES          # 6250
NT = (NLOC + 127) // 128    # 49 dst tiles per core
NLOC_PAD = NT * 128         # 6272
KSUP = 2
NSUP = (NT + KSUP - 1) // KSUP          # 25
SUP_TILES = [min(KSUP, NT - T * KSUP) for T in range(NSUP)]
NQ = 4
# AG chunk boundaries (tile-aligned rows within a core)
Q_TILES = [13, 12, 12, 12]              # tiles per chunk
Q_ROW0 = [0, 1664, 3200, 4736]          # first local row of chunk
Q_ROWS = [1664, 1536, 1536, 1514]       # local rows in chunk (last clipped)
Q_OFF8 = [0, 13312, 25600, 37888]       # chunk offset in the full table
BF = ml_dtypes.bfloat16


def _ts(i, size=128):
    return slice(i * size, (i + 1) * size)


def build_program(spec):
    """spec: dict with per-group layout (see prepare_inputs)."""
    ch = spec["ch"]                  # [NT][NQ] chunk counts
    cnt = spec["cnt"]                # [NT][NQ] valid counts
    slot_off = spec["slot_off"]      # [NT][NQ] slot base (per core)
    dl_off = spec["dl_off"]          # [NT][NQ] dl col base
    SLOTS = spec["slots"]            # total slots per core
    DLC = spec["dlc"]                # total dl cols
    CHMAX = spec["chmax"]

    nc = bacc.Bacc("TRN2", target_bir_lowering=False, debug=False,
                   num_devices=NCORES, num_swdge_queues=4)

    # ---- I/O ----
    xT = nc.dram_tensor("xT", [F_IN_PAD, NLOC_PAD], BF16, kind="ExternalInput")
    idx_d = nc.dram_tensor("idx", [128, SLOTS // 16], I16, kind="ExternalInput")
    dl_d = nc.dram_tensor("dl", [128, DLC], BF16, kind="ExternalInput")
    invdeg_d = nc.dram_tensor("invdeg", [128, NLOC_PAD], BF16, kind="ExternalInput")
    iota_d = nc.dram_tensor("iota", [128, 128], BF16, kind="ExternalInput")
    ident_d = nc.dram_tensor("ident", [128, 128], BF16, kind="ExternalInput")
    ident32_d = nc.dram_tensor("ident32", [128, 128], F32, kind="ExternalInput")
    wmap_d = nc.dram_tensor("wmap", [128, F_IN_PAD], BF16, kind="ExternalInput")
    bmap_d = nc.dram_tensor("bmap", [128, 1], F32, kind="ExternalInput")
    wl1_d = nc.dram_tensor("wl1", [128, D], BF16, kind="ExternalInput")
    wr1_d = nc.dram_tensor("wr1", [128, D], BF16, kind="ExternalInput")
    bl1_d = nc.dram_tensor("bl1", [128, 1], F32, kind="ExternalInput")
    wl2_d = nc.dram_tensor("wl2", [128, D2], BF16, kind="ExternalInput")
    wr2_d = nc.dram_tensor("wr2", [128, D2], BF16, kind="ExternalInput")
    bl2_d = nc.dram_tensor("bl2", [128, 2], F32, kind="ExternalInput")
    wl3_d = nc.dram_tensor("wl3", [128, 2 * W64], BF16, kind="ExternalInput")
    wr3_d = nc.dram_tensor("wr3", [128, 2 * W64], BF16, kind="ExternalInput")
    bl3_d = nc.dram_tensor("bl3", [W64, 1], F32, kind="ExternalInput")
    out_d = nc.dram_tensor("out", [NLOC, NCLS], F32, kind="ExternalOutput")

    # internal DRAM: per-quarter local slices + full tables
    locs = {}
    for nm in ("h0", "h1", "z3"):
        w = 128
        locs[nm] = [nc.dram_tensor(f"{nm}loc{q}", [Q_ROWS[q], w], BF16)
                    for q in range(NQ)]
    h0full = nc.dram_tensor("h0full", [N, D], BF16, addr_space="Shared")
    h1full = nc.dram_tensor("h1full", [N, D], BF16, addr_space="Shared")
    z3full = nc.dram_tensor("z3full", [N, 128], BF16, addr_space="Shared")

    groups = [list(range(NCORES))]
    self_queue = [0]

    def issue_ag(nm, full, q):
        nc.gpsimd.collective_compute(
            "AllGather", mybir.AluOpType.bypass, replica_groups=groups,
            ins=[locs[nm][q][:]],
            outs=[full[Q_OFF8[q]:Q_OFF8[q] + 8 * Q_ROWS[q], :]])

    with tile.TileContext(nc) as tc:
        with (
            tc.tile_pool(name="const", bufs=1) as cp,
            tc.tile_pool(name="hres", bufs=1) as hp,
            tc.tile_pool(name="gat", bufs=7) as gp,
            tc.tile_pool(name="sone", bufs=6) as sp,
            tc.tile_pool(name="work", bufs=3) as wp,
            tc.tile_pool(name="xin", bufs=4) as xp,
            tc.tile_pool(name="pa", bufs=2 * KSUP, space="PSUM") as pa,
            tc.tile_pool(name="po", bufs=2, space="PSUM") as po,
            tc.tile_pool(name="pt", bufs=2, space="PSUM") as pt,
        ):
            # ---- load constants ----
            idx_sb = cp.tile([128, SLOTS // 16], I16)
            dl_sb = cp.tile([128, DLC], BF16)
            invdeg = cp.tile([128, NLOC_PAD], BF16)
            iota = cp.tile([128, 128], BF16)
            ident = cp.tile([128, 128], BF16)
            ident32 = cp.tile([128, 128], F32)
            wmap = cp.tile([128, F_IN_PAD], BF16)
            bmap = cp.tile([128, 1], F32)
            wl1 = cp.tile([128, D], BF16)
            wr1 = cp.tile([128, D], BF16)
            bl1 = cp.tile([128, 1], F32)
            wl2 = cp.tile([128, D2], BF16)
            wr2 = cp.tile([128, D2], BF16)
            bl2 = cp.tile([128, 2], F32)
            wl3 = cp.tile([128, 2 * W64], BF16)
            wr3 = cp.tile([128, 2 * W64], BF16)
            bl3 = cp.tile([W64, 1], F32)
            for sb_t, dr in [(idx_sb, idx_d), (dl_sb, dl_d), (invdeg, invdeg_d),
                             (iota, iota_d), (ident, ident_d),
                             (ident32, ident32_d), (wmap, wmap_d),
                             (bmap, bmap_d), (wl1, wl1_d), (wr1, wr1_d),
                             (bl1, bl1_d), (wl2, wl2_d), (wr2, wr2_d),
                             (bl2, bl2_d), (wl3, wl3_d), (wr3, wr3_d),
                             (bl3, bl3_d)]:
                nc.scalar.dma_start(out=sb_t[:], in_=dr[:])

            # persistent transposed activations
            h0T = hp.tile([128, NLOC_PAD], BF16, tag="hA")
            h1T = hp.tile([128, NLOC_PAD], BF16, tag="hB")

            # zero gather-pool slots once (pads are reg-skipped -> stale;
            # 0 * S keeps matmul inputs finite)
            for zi in range(7):
                gz = gp.tile([128, KSUP * CHMAX, 128], BF16, tag="g",
                             name=f"gz{zi}")
                nc.vector.memset(gz[:], 0.0)

            def store_tile(t, srcT, nm):
                """PE-transpose column tile t of srcT -> node-major rows of
                the quarter loc tensors; returns after dma issue."""
                rows = min(128, NLOC - t * 128)
                ptr = pt.tile([128, 128], BF16, tag="pt")
                nc.tensor.transpose(ptr[:], srcT[:, _ts(t)], ident[:])
                nm_t = wp.tile([128, 128], BF16, tag="nm")
                nc.scalar.copy(nm_t[:], ptr[:])
                r0 = t * 128
                r1 = r0 + rows
                for q in range(NQ):
                    a, b = Q_ROW0[q], Q_ROW0[q] + Q_ROWS[q]
                    lo, hi = max(r0, a), min(r1, b)
                    if lo < hi:
                        nc.sync.dma_start(
                            out=locs[nm][q][lo - a:hi - a, :],
                            in_=nm_t[lo - r0:hi - r0, :])

            # ---- stage 0: h0T = W_map^T @ xT + b_map ----
            NH = NLOC_PAD // 2
            for half_n in (0, 1):
                slabs = [xp.tile([128, NH], BF16, tag="xs",
                                 name=f"xs{half_n}_{k}") for k in range(4)]
                for k in range(4):
                    nc.sync.dma_start(out=slabs[k][:],
                                      in_=xT[_ts(k), half_n * NH:(half_n + 1) * NH])
                nh_chunks = [(i * 512, min(512, NH - i * 512))
                             for i in range((NH + 511) // 512)]
                for n0, nw in nh_chunks:
                    ps = po.tile([128, 512], F32, tag="po")
                    for k in range(4):
                        nc.tensor.matmul(ps[:, 0:nw], wmap[:, _ts(k)],
                                         slabs[k][:, n0:n0 + nw],
                                         start=(k == 0), stop=(k == 3))
                    g0 = half_n * NH + n0
                    nc.vector.tensor_scalar(out=h0T[:, g0:g0 + nw],
                                            in0=ps[:, 0:nw],
                                            scalar1=bmap[:, 0:1], scalar2=None,
                                            op0=mybir.AluOpType.add)
            ag_tile = {12: 0, 24: 1, 36: 2, 48: 3}
            for t in range(NT):
                store_tile(t, h0T, "h0")
                if t in ag_tile:
                    issue_ag("h0", h0full, ag_tile[t])

            # ---- generic edge aggregation ----
            def aggregate(T, q, src_full, pa_tiles, done_ch, total_ch):
                """Gather + one-hot matmuls for supertile T, chunk q.
                done_ch[ti]: chunks already accumulated (for start/stop)."""
                ntl = SUP_TILES[T]
                g = gp.tile([128, KSUP * CHMAX, 128], BF16, tag="g")
                qa = Q_OFF8[q]
                qb = qa + 8 * Q_ROWS[q]
                chs = []
                for ti in range(ntl):
                    t = T * KSUP + ti
                    c_g, n_g = ch[t][q], cnt[t][q]
                    chs.append(c_g)
                    if c_g == 0:
                        continue
                    base = slot_off[t][q]
                    slots_g = c_g * 128
                    done = 0
                    gcol0 = ti * CHMAX
                    while done < slots_g:
                        n = min(1024, slots_g - done)
                        v = max(0, min(n_g - done, n))
                        if v == 0:
                            done += n
                            continue
                        nc.gpsimd.dma_gather(
                            g[:, gcol0 + done // 128:gcol0 + (done + n) // 128, :],
                            src_full[qa:qb, :],
                            idx_sb[:, (base + done) // 16:(base + done + n) // 16],
                            n, v, 128,
                            single_packet=True, queue_num=self_queue[0])
                        self_queue[0] = (self_queue[0] + 1) % 4
                        done += n
                ncols = sum(chs)
                if ncols == 0:
                    return
                sone = sp.tile([128, KSUP * CHMAX, 128], BF16, tag="S")
                d0 = dl_off[T * KSUP][q]
                io_b = iota[:].rearrange("p (o j) -> p o j", o=1) \
                    .broadcast_to([128, ncols, 128])
                dl_b = dl_sb[:, d0:d0 + ncols] \
                    .rearrange("p (c o) -> p c o", o=1) \
                    .broadcast_to([128, ncols, 128])
                nc.vector.tensor_tensor(out=sone[:, 0:ncols, :], in0=io_b,
                                        in1=dl_b, op=mybir.AluOpType.is_equal)
                scol = 0
                for ti in range(ntl):
                    c_g = chs[ti]
                    for cc in range(c_g):
                        nc.tensor.matmul(
                            pa_tiles[ti][:],
                            g[:, ti * CHMAX + cc, :], sone[:, scol + cc, :],
                            start=(done_ch[ti] == 0),
                            stop=(done_ch[ti] == total_ch[ti] - 1))
                        done_ch[ti] += 1
                    scol += c_g

            def layer_loop(src_full, tile_body, mean_dt=BF16, ag=None):
                """ag: (nm, full) to AllGather in quarters mid-loop."""
                boundary = {6: 0, 12: 1, 18: 2, 24: 3}
                for T in range(NSUP):
                    ntl = SUP_TILES[T]
                    pa_tiles = [pa.tile([128, 128], F32, tag="pa",
                                        name=f"pa_{T}_{i}")
                                for i in range(ntl)]
                    total_ch = [sum(ch[T * KSUP + ti][q] for q in range(NQ))
                                for ti in range(ntl)]
                    assert all(tc > 0 for tc in total_ch)
                    done_ch = [0] * ntl
                    for qi in range(NQ):
                        q = (T + qi) % NQ
                        aggregate(T, q, src_full, pa_tiles, done_ch, total_ch)
                    for ti in range(ntl):
                        t = T * KSUP + ti
                        mean = wp.tile([128, 128], mean_dt, tag="mean")
                        nc.vector.tensor_tensor(
                            out=mean[:], in0=pa_tiles[ti][:],
                            in1=invdeg[:, _ts(t)],
                            op=mybir.AluOpType.mult)
                        tile_body(t, mean)
                    if ag is not None and T in boundary:
                        issue_ag(ag[0], ag[1], boundary[T])

            # ---- layer 1 ----
            def l1_body(t, mean):
                p1 = po.tile([128, 128], F32, tag="po")
                nc.tensor.matmul(p1[:], wl1[:], mean[:], start=True, stop=False)
                nc.tensor.matmul(p1[:], wr1[:], h0T[:, _ts(t)],
                                 start=False, stop=True)
                nc.scalar.activation(out=h1T[:, _ts(t)], in_=p1[:],
                                     func=mybir.ActivationFunctionType.Relu,
                                     bias=bl1[:, 0:1], scale=1.0)
                store_tile(t, h1T, "h1")

            layer_loop(h0full, l1_body, ag=("h1", h1full))

            # ---- layer 2 (+ z3 projection) ----
            h2T0 = hp.tile([128, NLOC_PAD], BF16, tag="hA")  # reuses h0T slot
            h2T1 = hp.tile([128, NLOC_PAD], BF16, tag="hC")
            z3T = hp.tile([128, NLOC_PAD], BF16, tag="hD")   # rows 0:64 used
            nc.vector.memset(z3T[:], 0.0)

            def l2_body(t, mean):
                for hh, (h2T_h, wcol) in enumerate(((h2T0, _ts(0)),
                                                    (h2T1, _ts(1)))):
                    p2 = po.tile([128, 128], F32, tag="po")
                    nc.tensor.matmul(p2[:], wl2[:, wcol], mean[:],
                                     start=True, stop=False)
                    nc.tensor.matmul(p2[:], wr2[:, wcol], h1T[:, _ts(t)],
                                     start=False, stop=True)
                    nc.scalar.activation(
                        out=h2T_h[:, _ts(t)], in_=p2[:],
                        func=mybir.ActivationFunctionType.Relu,
                        bias=bl2[:, hh:hh + 1], scale=1.0)
                # z3 = h2 @ Wl3 (transposed: z3T = Wl3^T @ h2T), 64-wide
                pz = po.tile([128, 128], F32, tag="po")
                nc.tensor.matmul(pz[0:W64, :], wl3[:, 0:W64],
                                 h2T0[:, _ts(t)], start=True, stop=False)
                nc.tensor.matmul(pz[0:W64, :], wl3[:, W64:2 * W64],
                                 h2T1[:, _ts(t)], start=False, stop=True)
                nc.scalar.copy(z3T[0:W64, _ts(t)], pz[0:W64, :])
                store_tile(t, z3T, "z3")

            layer_loop(h1full, l2_body, ag=("z3", z3full))

            # ---- layer 3 + log_softmax ----
            def l3_body(t, mean):
                rows = min(128, NLOC - t * 128)
                p3 = po.tile([128, 128], F32, tag="po")
                nc.tensor.matmul(p3[0:W64, :], wr3[:, 0:W64],
                                 h2T0[:, _ts(t)], start=True, stop=False)
                nc.tensor.matmul(p3[0:W64, :], wr3[:, W64:2 * W64],
                                 h2T1[:, _ts(t)], start=False, stop=True)
                comb = wp.tile([W64, 128], F32, tag="comb")
                nc.vector.tensor_tensor(out=comb[:], in0=mean[0:W64, :],
                                        in1=p3[0:W64, :],
                                        op=mybir.AluOpType.add)
                comb2 = wp.tile([W64, 128], F32, tag="comb2")
                nc.scalar.activation(out=comb2[:], in_=comb[:],
                                     func=mybir.ActivationFunctionType.Identity,
                                     bias=bl3[0:W64, 0:1], scale=1.0)
                ptf = pt.tile([128, 128], F32, tag="pt")
                nc.tensor.transpose(ptf[:, 0:W64], comb2[:],
                                    ident32[0:W64, 0:W64])
                xm = wp.tile([128, 1], F32, tag="xm")
                nc.vector.tensor_reduce(out=xm[:], in_=ptf[:, 0:NCLS],
                                        axis=mybir.AxisListType.X,
                                        op=mybir.AluOpType.max, negate=True)
                tt = wp.tile([128, NCLS], F32, tag="tt")
                nc.scalar.activation(out=tt[:], in_=ptf[:, 0:NCLS],
                                     func=mybir.ActivationFunctionType.Identity,
                                     bias=xm[:, 0:1], scale=1.0)
                ex = wp.tile([128, NCLS], F32, tag="ex")
                ssum = wp.tile([128, 1], F32, tag="ssum")
                nc.scalar.activation(out=ex[:], in_=tt[:],
                                     func=mybir.ActivationFunctionType.Exp,
                                     accum_out=ssum[:])
                lse = wp.tile([128, 1], F32, tag="lse")
                nc.scalar.activation(out=lse[:], in_=ssum[:],
                                     func=mybir.ActivationFunctionType.Ln)
                lsn = wp.tile([128, 1], F32, tag="lsn")
                nc.scalar.mul(lsn[:], lse[:], -1.0)
                fin = wp.tile([128, NCLS], F32, tag="fin")
                nc.scalar.activation(out=fin[:], in_=tt[:],
                                     func=mybir.ActivationFunctionType.Identity,
                                     bias=lsn[:, 0:1], scale=1.0)
                nc.sync.dma_start(out=out_d[t * 128: t * 128 + rows, :],
                                  in_=fin[0:rows, :])

            layer_loop(z3full, l3_body, mean_dt=F32)

    nc.compile()
    return nc


# ---------------- host side ----------------

def _pack_idx_groups(vals_per_group):
    """Each group's int16 slot array -> [128, L//16] wrap-16 packing, x8."""
    cols = []
    for vals in vals_per_group:
        L = vals.shape[0]
        if L == 0:
            continue
        arr = vals.reshape(L // 16, 16).T
        cols.append(np.tile(arr, (8, 1)))
    return np.ascontiguousarray(np.concatenate(cols, axis=1))


def prepare_inputs(x, edge_index, W_map, b_map, Wl1, bl1, Wr1, Wl2, bl2, Wr2,
                   Wl3, bl3, Wr3):
    src = np.asarray(edge_index[0], dtype=np.int64)
    dst = np.asarray(edge_index[1], dtype=np.int64)

    core = dst // NLOC
    local = dst - core * NLOC
    t_loc = local >> 7
    dloc = local & 127
    # src chunk + offset within the AG'd table layout
    c_src = src // NLOC
    r_src = src - c_src * NLOC
    q_src = np.searchsorted(np.array(Q_ROW0[1:]), r_src, side="right")
    q_rows = np.array(Q_ROWS)[q_src]
    idx16 = (c_src * q_rows + (r_src - np.array(Q_ROW0)[q_src])).astype(np.int16)

    # fine group (core, tile, q)
    fine = (core * NT + t_loc) * NQ + q_src
    NFINE = NCORES * NT * NQ
    counts = np.bincount(fine, minlength=NFINE).reshape(NCORES, NT, NQ)
    # shared chunk layout: max over cores so the SPMD program is uniform
    cnt_max = counts.max(axis=0)                    # [NT][NQ]
    ch = np.ceil(cnt_max / 128).astype(np.int64)    # [NT][NQ] chunks
    CHMAX = int(ch.max())
    slots_g = ch * 128
    # slot base per (t, q) in (T-major, q, ti) order
    slot_off = np.zeros((NT, NQ), np.int64)
    dl_off = np.zeros((NT, NQ), np.int64)
    pos_s = 0
    pos_d = 0
    for T in range(NSUP):
        for q in range(NQ):
            for ti in range(SUP_TILES[T]):
                t = T * KSUP + ti
                slot_off[t][q] = pos_s
                dl_off[t][q] = pos_d
                pos_s += slots_g[t][q]
                pos_d += ch[t][q]
    SLOTS = int(pos_s)
    DLC = int(pos_d)

    order = np.argsort(fine, kind="stable")
    fine_s = fine[order]
    offs = np.concatenate([[0], np.cumsum(counts.reshape(-1))])
    pos = np.arange(E) - np.repeat(offs[:-1], counts.reshape(-1))

    fine_base = np.zeros(NFINE, np.int64)
    for c in range(NCORES):
        for t in range(NT):
            for q in range(NQ):
                fine_base[(c * NT + t) * NQ + q] = c * SLOTS + slot_off[t][q]
    slot = fine_base[fine_s] + pos

    big_idx = np.full(NCORES * SLOTS, -1, dtype=np.int16)
    big_dl = np.full(NCORES * SLOTS, 999.0, dtype=np.float32)
    big_idx[slot] = idx16[order]
    big_dl[slot] = dloc[order].astype(np.float32)
    big_idx = big_idx.reshape(NCORES, SLOTS)
    big_dl = big_dl.reshape(NCORES, SLOTS)
    # uniform valid counts across cores: pad [cnt_c, cnt_max) with idx 0
    # (real descriptor, harmless row; dl stays 999 -> S=0).  Slots beyond
    # cnt_max keep idx=-1 (descriptor-gen skips them).
    for c in range(NCORES):
        for t in range(NT):
            for q in range(NQ):
                a = slot_off[t][q]
                c_c = counts[c][t][q]
                c_m = cnt_max[t][q]
                if c_c < c_m:
                    big_idx[c, a + c_c:a + c_m] = 0

    cnt_arr = [[int(cnt_max[t][q]) for q in range(NQ)] for t in range(NT)]
    ch_arr = [[int(ch[t][q]) for q in range(NQ)] for t in range(NT)]
    spec = {"ch": ch_arr, "cnt": cnt_arr,
            "slot_off": slot_off.tolist(), "dl_off": dl_off.tolist(),
            "slots": SLOTS, "dlc": DLC, "chmax": CHMAX}

    # degrees
    cnt_n = np.bincount(dst, minlength=N).astype(np.float32)
    inv = 1.0 / np.maximum(cnt_n, 1.0)

    Wmap_pad = np.zeros((F_IN_PAD, 128), np.float32)
    Wmap_pad[0:F_IN] = W_map
    wmap_kt = np.concatenate([Wmap_pad[_ts(k)] for k in range(4)], axis=1)
    Wl3_pad = np.zeros((D2, W64), np.float32)
    Wl3_pad[:, 0:NCLS] = Wl3
    wl3_kt = np.concatenate([Wl3_pad[_ts(k)] for k in range(2)], axis=1)
    Wr3_pad = np.zeros((D2, W64), np.float32)
    Wr3_pad[:, 0:NCLS] = Wr3
    wr3_kt = np.concatenate([Wr3_pad[_ts(k)] for k in range(2)], axis=1)
    bl3_pad = np.zeros((W64, 1), np.float32)
    bl3_pad[0:NCLS, 0] = bl3

    shared = {
        "iota": np.ascontiguousarray(
            np.tile(np.arange(128, dtype=np.float32), (128, 1))).astype(BF),
        "ident": np.eye(128, dtype=np.float32).astype(BF),
        "ident32": np.eye(128, dtype=np.float32),
        "wmap": np.ascontiguousarray(wmap_kt).astype(BF),
        "bmap": np.ascontiguousarray(b_map.reshape(128, 1)),
        "wl1": np.ascontiguousarray(Wl1).astype(BF),
        "wr1": np.ascontiguousarray(Wr1).astype(BF),
        "bl1": np.ascontiguousarray(bl1.reshape(128, 1)),
        "wl2": np.ascontiguousarray(Wl2).astype(BF),
        "wr2": np.ascontiguousarray(Wr2).astype(BF),
        "bl2": np.ascontiguousarray(bl2.reshape(2, 128).T),
        "wl3": np.ascontiguousarray(wl3_kt).astype(BF),
        "wr3": np.ascontiguousarray(wr3_kt).astype(BF),
        "bl3": bl3_pad,
    }

    in_maps = []
    for c in range(NCORES):
        xT_pad = np.zeros((F_IN_PAD, NLOC_PAD), np.float32)
        xT_pad[0:F_IN, 0:NLOC] = x[c * NLOC:(c + 1) * NLOC].T
        xT_pad = xT_pad.astype(BF)

        groups_vals = []
        dl_cols = []
        for T in range(NSUP):
            for q in range(NQ):
                for ti in range(SUP_TILES[T]):
                    t = T * KSUP + ti
                    a = slot_off[t][q]
                    b = a + slots_g[t][q]
                    groups_vals.append(big_idx[c, a:b])
                    if b > a:
                        dl_cols.append(big_dl[c, a:b].reshape(-1, 128).T)
        idx_arr = _pack_idx_groups(groups_vals)
        dl_arr = np.ascontiguousarray(
            np.concatenate(dl_cols, axis=1)).astype(BF)

        inv_pad = np.ones(NLOC_PAD, np.float32)
        inv_pad[0:NLOC] = inv[c * NLOC:(c + 1) * NLOC]
        invdeg_arr = np.ascontiguousarray(
            np.broadcast_to(inv_pad, (128, NLOC_PAD))).astype(BF)

        m = {"xT": xT_pad, "idx": idx_arr, "dl": dl_arr,
             "invdeg": invdeg_arr}
        m.update(shared)
        in_maps.append(m)
    return in_maps, spec


_prog_cache = {}


def kernel(**inputs) -> np.ndarray:
    args = {k: np.asarray(v) for k, v in inputs.items()}
    in_maps, spec = prepare_inputs(
        args["x"], args["edge_index"], args["W_map"], args["b_map"],
        args["Wl1"], args["bl1"], args["Wr1"], args["Wl2"], args["bl2"],
        args["Wr2"], args["Wl3"], args["bl3"], args["Wr3"])

    key = (spec["slots"], spec["dlc"],
           tuple(tuple(r) for r in spec["ch"]),
           tuple(tuple(r) for r in spec["cnt"]))
    if key not in _prog_cache:
        _prog_cache[key] = build_program(spec)
    nc = _prog_cache[key]

    trace = os.environ.get("KERNEL_TRACE", "0") == "1"
    kw = {}
    if trace:
        import concourse.bass_utils as bu
        bu.upload_artifacts = lambda t: ""
        kw = dict(trace=True, tmpdir=os.environ.get(
            "KERNEL_TRACE_DIR", "/tmp/kernel_trace"))
    res = run_bass_kernel_spmd(nc, in_maps, list(range(NCORES)), **kw)
    if trace and res.exec_time_ns is not None:
        print(f"HW exec time: {res.exec_time_ns} ns")

    out = np.concatenate([res.results[c]["out"] for c in range(NCORES)], axis=0)
    return out.astype(np.float32)
